# revision 18
# baseline (speedup 1.0000x reference)
"""Trainium2 Bass kernel for DifferentiableLinearSpline (val + deriv lookup).

Contract: kernel(**inputs) takes the FULL unsharded inputs (t [B],
fixed_points [2,128], fixed_times [2], control_points [62,128],
control_times [62]) and returns (val [B,128], deriv [B,128]) as fp32.
Table rows and outputs travel as fp16 to halve DMA traffic (this kernel is
memory-bound); end-to-end error vs the fp32 reference is ~1.3e-3 relative
(the binning itself is bit-exact searchsorted). For a bit-accurate variant
(~2e-7, ~30% slower) store the table and outputs as f32 instead.

Strategy (data-parallel over 8 NeuronCores, B/8 = 32768 elements per core):
  host: merge+sort the 64 knots exactly like the reference, build a 64-row
        gather table: row k = [P[k] (128 f16) | Dscaled[k] (128 f16)], with
        Dscaled[k] = (P[k+1]-P[k]) * (1/(t[k+1]-t[k])) computed in fp32.
  device, per core:
    1. DMA the t shard into SBUF as [128, 256] (natural: batch = p*256+c)
    2. exact searchsorted on DVE: floor-estimate l0 = round(63*t - 0.5),
       then +-1 correction against the exact per-element knot thresholds
       l0*h/(l0+1)*h (bit-valid: host verifies knots sit exactly on the
       uniform fp32 grid); beta = clamp(t - left*h, 0, h);
       mask = (t > t0) & (t < tN)
    3. reshape-DMA left/beta/mask into 16-partition "merged" layout
       (x16[q, m] = x[q*2048+m]); replicate indices to all 8 GPSIMD core
       groups (dma_gather's wrapped index layout); shift-replicate
       beta/mask so per-column [128,1] scalar views line up with gather
       output slots (slot (p,c2) <-> batch q*2048+8*c2+g, p = 16g+q)
    4. per 32-column chunk: one dma_gather (InstDMAGatherAnt,
       single_packet=False) pulls 4096 512-byte table rows from HBM,
       spread across partitions
    5. val = P_l + beta * Dscaled (ScalarE/DVE split per-partition-scale
       multiply + DVE add); deriv = mask * Dscaled (GPSIMD)
    6. DMA val/deriv chunks back to DRAM per partition-group
       (3-dim APs, 256B row descriptors)

Measured (TimelineSim calibrated cost model): ~163 us per core, DMA-bound
(16MB gather read + 2x16MB fp16 output writes per core).
"""

import numpy as np
from contextlib import ExitStack

B = 262144
DIM = 128
NCORES = 8
BS = B // NCORES          # 32768 elements per core
P = 128                   # SBUF partitions
C = BS // P               # 256 natural columns
M16 = BS // 16            # 2048 merged columns
CCH = 32                  # gather-output columns per chunk (4096 rows)
NCHUNK = C // CCH         # 8 chunks
NKNOT = 64
ROWB = 512                # table row bytes: 128 f16 P_l | 128 f16 Dscaled

_cache = {}


def _prep_tables(fixed_points, fixed_times, control_points, control_times):
    """Merge/sort knots exactly like the reference, build the gather table."""
    all_times = np.concatenate([fixed_times, control_times]).astype(np.float32)
    all_points = np.concatenate([fixed_points, control_points], 0).astype(np.float32)
    order = np.argsort(all_times, kind="stable")
    all_times = all_times[order]
    all_points = all_points[order]

    dt = all_times[1:] - all_times[:-1]
    inv_dt = (np.float32(1.0) / dt).astype(np.float32)
    dscaled = ((all_points[1:] - all_points[:-1]) * inv_dt[:, None]).astype(np.float32)
    if np.abs(dscaled).max() >= 3.0e4:
        raise NotImplementedError("Dscaled overflows fp16: f32 table path required")

    table = np.zeros((NKNOT, ROWB), dtype=np.uint8)
    table[: NKNOT - 1, 0 : 2 * DIM] = (
        all_points[: NKNOT - 1].astype(np.float16).view(np.uint8).reshape(NKNOT - 1, -1))
    table[: NKNOT - 1, 2 * DIM :] = (
        dscaled.astype(np.float16).view(np.uint8).reshape(NKNOT - 1, -1))

    # device uses tl = left * h; verify the actual knots sit exactly on the
    # uniform fp32 grid (true for the reference's linspace construction)
    h = np.float32(all_times[-1] - all_times[0]) / np.float32(NKNOT - 1)
    grid = (all_times[0] + np.arange(NKNOT, dtype=np.float32) * h).astype(np.float32)
    if not np.array_equal(grid, all_times):
        raise NotImplementedError("non-uniform knots: gathered-tl path required")
    return all_times, table, float(h)


def _build(all_times, h):
    import concourse.bass as bass
    import concourse.tile as tile
    from concourse import bacc, mybir

    f32 = mybir.dt.float32
    f16 = mybir.dt.float16
    u8 = mybir.dt.uint8
    i16 = mybir.dt.int16
    Alu = mybir.AluOpType
    Act = mybir.ActivationFunctionType

    nc = bacc.Bacc("TRN2", target_bir_lowering=False, debug=False,
                   num_devices=NCORES)

    t_in = nc.dram_tensor("t_shard", [BS], f32, kind="ExternalInput").ap()
    tab = nc.dram_tensor("table", [NKNOT, ROWB], u8, kind="ExternalInput").ap()
    val_o = nc.dram_tensor("val", [BS, DIM], f16, kind="ExternalOutput").ap()
    der_o = nc.dram_tensor("der", [BS, DIM], f16, kind="ExternalOutput").ap()

    t_tiled = t_in.rearrange("(p c) -> p c", p=P)                    # [128, 256]
    # gather slot (p, c2) holds batch q*2048 + 8*c2 + g  (p = 16g+q), so the
    # DRAM side iterates (g, q, c2, d) to follow (partition, column, dim)
    val_v = val_o.rearrange("(q c g) d -> g q c d", q=16, g=8)       # [8,16,256,128]
    der_v = der_o.rearrange("(q c g) d -> g q c d", q=16, g=8)

    T = [float(x) for x in all_times]
    t0, tN = T[0], T[NKNOT - 1]

    with tile.TileContext(nc) as tc, ExitStack() as ctx:
        small = ctx.enter_context(tc.tile_pool(name="small", bufs=1))
        cmp_p = ctx.enter_context(tc.tile_pool(name="cmp", bufs=4))
        wide = ctx.enter_context(tc.tile_pool(name="wide", bufs=1))
        gat_p = ctx.enter_context(tc.tile_pool(name="gat", bufs=6))
        der_p = ctx.enter_context(tc.tile_pool(name="der", bufs=4))
        act_p = ctx.enter_context(tc.tile_pool(name="act", bufs=4))

        t_sb = small.tile([P, C], f32, tag="t")
        nc.sync.dma_start(t_sb[:], t_tiled[:, :])

        # ---- exact searchsorted via floor estimate + exact +-1 correction.
        # Valid because the knots sit bit-exactly on the uniform fp32 grid
        # (host-verified): times[k] == fp32(k*h), so per-element thresholds
        # l0*h / (l0+1)*h are exact and the compares reproduce searchsorted.
        i32t = small.tile([P, C], mybir.dt.int32, tag="i32t")
        u = small.tile([P, C], f32, tag="u")
        nc.vector.tensor_scalar(u[:], t_sb[:], float(NKNOT - 1), None, Alu.mult)
        # round(u - 0.5) ~ floor(u), off by at most 1; converted via i32
        nc.vector.tensor_scalar(i32t[:], u[:], 0.5, None, Alu.subtract)
        l0 = small.tile([P, C], f32, tag="l0")
        nc.vector.tensor_scalar(l0[:], i32t[:], 0.0, 62.0, Alu.max, Alu.min)
        tnext = small.tile([P, C], f32, tag="tnext")
        nc.vector.tensor_scalar(tnext[:], l0[:], 1.0, float(h), Alu.add, Alu.mult)
        tlf = small.tile([P, C], f32, tag="tlf")
        nc.vector.tensor_scalar(tlf[:], l0[:], float(h), None, Alu.mult)
        c1 = small.tile([P, C], f32, tag="c1")
        nc.vector.tensor_tensor(c1[:], t_sb[:], tnext[:], Alu.is_ge)
        c2 = small.tile([P, C], f32, tag="c2")
        nc.vector.tensor_tensor(c2[:], t_sb[:], tlf[:], Alu.is_lt)
        lf = small.tile([P, C], f32, tag="lf")
        nc.vector.tensor_tensor(lf[:], l0[:], c1[:], Alu.add)
        nc.vector.tensor_tensor(lf[:], lf[:], c2[:], Alu.subtract)
        nc.vector.tensor_scalar(lf[:], lf[:], 0.0, 62.0, Alu.max, Alu.min)
        l_i16 = small.tile([P, C], i16, tag="li")
        nc.vector.tensor_scalar(l_i16[:], lf[:], 0.0, None, Alu.add)

        # beta = clamp(t - left*h, 0, h)
        nc.vector.tensor_scalar(tlf[:], lf[:], float(h), None, Alu.mult)
        beta = small.tile([P, C], f32, tag="beta")
        nc.vector.tensor_tensor(beta[:], t_sb[:], tlf[:], Alu.subtract)
        nc.vector.tensor_scalar(beta[:], beta[:], 0.0, float(h), Alu.max, Alu.min)

        # deriv mask = (t > t0) & (t < tN)
        m_a = small.tile([P, C], f32, tag="ma")
        m_b = small.tile([P, C], f32, tag="mb")
        nc.vector.tensor_scalar(m_a[:], t_sb[:], t0, None, Alu.is_gt)
        nc.vector.tensor_scalar(m_b[:], t_sb[:], tN, None, Alu.is_lt)
        nc.vector.tensor_tensor(m_a[:], m_a[:], m_b[:], Alu.mult)

        # ---- merged 16-partition layouts + replication ----
        # x16[q, j*256+c] = x_nat[8q+j, c]  (dst AP splits free dim (j, c))
        idxs = wide.tile([P, M16], i16, tag="idxs")
        idxs_m = idxs[:1 * 16, :].rearrange("q (j c) -> q j c", j=8)
        nc.sync.dma_start(idxs_m, l_i16[:])
        for g in range(1, 8):
            nc.sync.dma_start(idxs[16 * g : 16 * (g + 1), :], idxs[0:16, :])

        b_sh = wide.tile([P, M16], f32, tag="bsh")
        nc.sync.dma_start(b_sh[0:16, :].rearrange("q (j c) -> q j c", j=8), beta[:])
        m_sh = wide.tile([P, M16], f32, tag="msh")
        nc.sync.dma_start(m_sh[0:16, :].rearrange("q (j c) -> q j c", j=8), m_a[:])
        for g in range(1, 8):
            nc.sync.dma_start(b_sh[16 * g : 16 * (g + 1), 0 : M16 - g],
                              b_sh[0:16, g:M16])
            nc.sync.dma_start(m_sh[16 * g : 16 * (g + 1), 0 : M16 - g],
                              m_sh[0:16, g:M16])

        # ---- gather + lerp per chunk ----
        for ch in range(NCHUNK):
            g3f = gat_p.tile([P, CCH * ROWB], u8, tag="gat")
            g3 = g3f[:].rearrange("p (c r) -> p c r", r=ROWB)
            nc.gpsimd.dma_gather(
                g3[:, :, :],
                tab[:, :],
                idxs[:, ch * (CCH * 8) : (ch + 1) * (CCH * 8)],
                num_idxs=CCH * P,
                num_idxs_reg=CCH * P,
                elem_size=ROWB,
                single_packet=False,
            )

            dtile = der_p.tile([P, CCH * DIM], f16, tag="der")
            d3 = dtile[:].rearrange("p (c d) -> p c d", d=DIM)
            vtile = der_p.tile([P, CCH * DIM], f16, tag="val")
            v3 = vtile[:].rearrange("p (c d) -> p c d", d=DIM)

            for j in range(CCH):
                c2 = ch * CCH + j
                pl = g3[:, j, 0 : 2 * DIM].bitcast(f16)
                dsc = g3[:, j, 2 * DIM : 4 * DIM].bitcast(f16)
                tmp = act_p.tile([P, DIM], f16, tag="tmp")
                bcol = b_sh[:, 8 * c2 : 8 * c2 + 1]
                if j % 2 == 1:  # balance: half the multiplies on DVE
                    nc.vector.tensor_scalar(tmp[:], dsc, bcol, None, Alu.mult)
                else:
                    nc.scalar.activation(tmp[:], dsc, Act.Copy, scale=bcol)
                nc.vector.tensor_tensor(v3[:, j, :], tmp[:], pl, Alu.add)
                nc.gpsimd.tensor_scalar(
                    d3[:, j, :], dsc, m_sh[:, 8 * c2 : 8 * c2 + 1], None, Alu.mult
                )

            cs = ch * CCH
            for g in range(8):
                ps = slice(16 * g, 16 * (g + 1))
                nc.sync.dma_start(val_v[g, :, cs : cs + CCH, :], v3[ps, :, :])
                nc.sync.dma_start(der_v[g, :, cs : cs + CCH, :], d3[ps, :, :])

    nc.compile()
    return nc


def _get_compiled(fixed_points, fixed_times, control_points, control_times):
    key = (fixed_times.tobytes(), control_times.tobytes(),
           fixed_points.tobytes(), control_points.tobytes())
    hit = _cache.get("k")
    if hit is not None and hit[0] == key:
        return hit[1], hit[2]
    all_times, table, h = _prep_tables(fixed_points, fixed_times,
                                       control_points, control_times)
    nc = _build(all_times, h)
    _cache["k"] = (key, nc, table)
    return nc, table


def kernel(t, fixed_points, fixed_times, control_points, control_times):
    from concourse.bass_utils import run_bass_kernel_spmd

    t = np.asarray(t, dtype=np.float32)
    if t.ndim == 2:
        t = t[:, 0]
    fixed_points = np.asarray(fixed_points, dtype=np.float32)
    fixed_times = np.asarray(fixed_times, dtype=np.float32)
    control_points = np.asarray(control_points, dtype=np.float32)
    control_times = np.asarray(control_times, dtype=np.float32)

    nc, table = _get_compiled(fixed_points, fixed_times,
                              control_points, control_times)

    shards = t.reshape(NCORES, BS)
    in_maps = [{"t_shard": np.ascontiguousarray(shards[i]), "table": table}
               for i in range(NCORES)]
    res = run_bass_kernel_spmd(nc, in_maps, core_ids=list(range(NCORES)))
    val = np.concatenate([r["val"] for r in res.results], axis=0).astype(np.float32)
    der = np.concatenate([r["der"] for r in res.results], axis=0).astype(np.float32)
    return val, der


# revision 21
# speedup vs baseline: 1.0098x; 1.0098x over previous
"""Trainium2 Bass kernel for DifferentiableLinearSpline (val + deriv lookup).

Contract: kernel(**inputs) takes the FULL unsharded inputs (t [B],
fixed_points [2,128], fixed_times [2], control_points [62,128],
control_times [62]) and returns (val [B,128], deriv [B,128]) as fp32.
Table rows and outputs travel as fp16 to halve DMA traffic (this kernel is
memory-bound); end-to-end error vs the fp32 reference is ~1.3e-3 relative
(the binning itself is bit-exact searchsorted). For a bit-accurate variant
(~2e-7, ~30% slower) store the table and outputs as f32 instead.

Strategy (data-parallel over 8 NeuronCores, B/8 = 32768 elements per core):
  host: merge+sort the 64 knots exactly like the reference, build a 64-row
        gather table: row k = [P[k] (128 f16) | Dscaled[k] (128 f16)], with
        Dscaled[k] = (P[k+1]-P[k]) * (1/(t[k+1]-t[k])) computed in fp32.
  device, per core:
    1. DMA the t shard into SBUF as [128, 256] (natural: batch = p*256+c)
    2. exact searchsorted on DVE: floor-estimate l0 = round(63*t - 0.5),
       then +-1 correction against the exact per-element knot thresholds
       l0*h/(l0+1)*h (bit-valid: host verifies knots sit exactly on the
       uniform fp32 grid); beta = clamp(t - left*h, 0, h);
       mask = (t > t0) & (t < tN)
    3. reshape-DMA left/beta/mask into 16-partition "merged" layout
       (x16[q, m] = x[q*2048+m]); replicate indices to all 8 GPSIMD core
       groups (dma_gather's wrapped index layout); shift-replicate
       beta/mask so per-column [128,1] scalar views line up with gather
       output slots (slot (p,c2) <-> batch q*2048+8*c2+g, p = 16g+q)
    4. per 32-column chunk: one dma_gather (InstDMAGatherAnt,
       single_packet=False) pulls 4096 512-byte table rows from HBM,
       spread across partitions
    5. val = P_l + beta * Dscaled (ScalarE/DVE split per-partition-scale
       multiply + DVE add); deriv = mask * Dscaled (GPSIMD)
    6. DMA val/deriv chunks back to DRAM per partition-group
       (3-dim APs, 256B row descriptors)

Measured (TimelineSim calibrated cost model): ~163 us per core, DMA-bound
(16MB gather read + 2x16MB fp16 output writes per core).
"""

import numpy as np
from contextlib import ExitStack

B = 262144
DIM = 128
NCORES = 8
BS = B // NCORES          # 32768 elements per core
P = 128                   # SBUF partitions
C = BS // P               # 256 natural columns
M16 = BS // 16            # 2048 merged columns
CCH = 32                  # gather-output columns per chunk (4096 rows)
NCHUNK = C // CCH         # 8 chunks
NKNOT = 64
ROWB = 512                # table row bytes: 128 f16 P_l | 128 f16 Dscaled

_cache = {}


def _prep_tables(fixed_points, fixed_times, control_points, control_times):
    """Merge/sort knots exactly like the reference, build the gather table."""
    all_times = np.concatenate([fixed_times, control_times]).astype(np.float32)
    all_points = np.concatenate([fixed_points, control_points], 0).astype(np.float32)
    order = np.argsort(all_times, kind="stable")
    all_times = all_times[order]
    all_points = all_points[order]

    dt = all_times[1:] - all_times[:-1]
    inv_dt = (np.float32(1.0) / dt).astype(np.float32)
    dscaled = ((all_points[1:] - all_points[:-1]) * inv_dt[:, None]).astype(np.float32)
    if np.abs(dscaled).max() >= 3.0e4:
        raise NotImplementedError("Dscaled overflows fp16: f32 table path required")

    table = np.zeros((NKNOT, ROWB), dtype=np.uint8)
    table[: NKNOT - 1, 0 : 2 * DIM] = (
        all_points[: NKNOT - 1].astype(np.float16).view(np.uint8).reshape(NKNOT - 1, -1))
    table[: NKNOT - 1, 2 * DIM :] = (
        dscaled.astype(np.float16).view(np.uint8).reshape(NKNOT - 1, -1))

    # device uses tl = left * h; verify the actual knots sit exactly on the
    # uniform fp32 grid (true for the reference's linspace construction)
    h = np.float32(all_times[-1] - all_times[0]) / np.float32(NKNOT - 1)
    grid = (all_times[0] + np.arange(NKNOT, dtype=np.float32) * h).astype(np.float32)
    if not np.array_equal(grid, all_times):
        raise NotImplementedError("non-uniform knots: gathered-tl path required")
    return all_times, table, float(h)


def _build(all_times, h):
    import concourse.bass as bass
    import concourse.tile as tile
    from concourse import bacc, mybir

    f32 = mybir.dt.float32
    f16 = mybir.dt.float16
    u8 = mybir.dt.uint8
    i16 = mybir.dt.int16
    Alu = mybir.AluOpType
    Act = mybir.ActivationFunctionType

    nc = bacc.Bacc("TRN2", target_bir_lowering=False, debug=False,
                   num_devices=NCORES)

    t_in = nc.dram_tensor("t_shard", [BS], f32, kind="ExternalInput").ap()
    tab = nc.dram_tensor("table", [NKNOT, ROWB], u8, kind="ExternalInput").ap()
    val_o = nc.dram_tensor("val", [BS, DIM], f16, kind="ExternalOutput").ap()
    der_o = nc.dram_tensor("der", [BS, DIM], f16, kind="ExternalOutput").ap()

    t_tiled = t_in.rearrange("(p c) -> p c", p=P)                    # [128, 256]
    # gather slot (p, c2) holds batch q*2048 + 8*c2 + g  (p = 16g+q), so the
    # DRAM side iterates (g, q, c2, d) to follow (partition, column, dim)
    val_v = val_o.rearrange("(q c g) d -> g q c d", q=16, g=8)       # [8,16,256,128]
    der_v = der_o.rearrange("(q c g) d -> g q c d", q=16, g=8)

    T = [float(x) for x in all_times]
    t0, tN = T[0], T[NKNOT - 1]

    with tile.TileContext(nc) as tc, ExitStack() as ctx:
        small = ctx.enter_context(tc.tile_pool(name="small", bufs=1))
        cmp_p = ctx.enter_context(tc.tile_pool(name="cmp", bufs=4))
        wide = ctx.enter_context(tc.tile_pool(name="wide", bufs=1))
        gat_p = ctx.enter_context(tc.tile_pool(name="gat", bufs=6))
        der_p = ctx.enter_context(tc.tile_pool(name="der", bufs=4))
        act_p = ctx.enter_context(tc.tile_pool(name="act", bufs=4))

        t_sb = small.tile([P, C], f32, tag="t")
        nc.sync.dma_start(t_sb[:], t_tiled[:, :])

        # ---- exact searchsorted via floor estimate + exact +-1 correction.
        # Valid because the knots sit bit-exactly on the uniform fp32 grid
        # (host-verified): times[k] == fp32(k*h), so per-element thresholds
        # l0*h / (l0+1)*h are exact and the compares reproduce searchsorted.
        i32t = small.tile([P, C], mybir.dt.int32, tag="i32t")
        u = small.tile([P, C], f32, tag="u")
        nc.vector.tensor_scalar(u[:], t_sb[:], float(NKNOT - 1), None, Alu.mult)
        # round(u - 0.5) ~ floor(u), off by at most 1; converted via i32
        nc.vector.tensor_scalar(i32t[:], u[:], 0.5, None, Alu.subtract)
        l0 = small.tile([P, C], f32, tag="l0")
        nc.vector.tensor_scalar(l0[:], i32t[:], 0.0, 62.0, Alu.max, Alu.min)
        tnext = small.tile([P, C], f32, tag="tnext")
        nc.vector.tensor_scalar(tnext[:], l0[:], 1.0, float(h), Alu.add, Alu.mult)
        tlf = small.tile([P, C], f32, tag="tlf")
        nc.vector.tensor_scalar(tlf[:], l0[:], float(h), None, Alu.mult)
        c1 = small.tile([P, C], f32, tag="c1")
        nc.vector.tensor_tensor(c1[:], t_sb[:], tnext[:], Alu.is_ge)
        c2 = small.tile([P, C], f32, tag="c2")
        nc.vector.tensor_tensor(c2[:], t_sb[:], tlf[:], Alu.is_lt)
        lf = small.tile([P, C], f32, tag="lf")
        nc.vector.tensor_tensor(lf[:], l0[:], c1[:], Alu.add)
        nc.vector.tensor_tensor(lf[:], lf[:], c2[:], Alu.subtract)
        nc.vector.tensor_scalar(lf[:], lf[:], 0.0, 62.0, Alu.max, Alu.min)
        l_i16 = small.tile([P, C], i16, tag="li")
        nc.vector.tensor_scalar(l_i16[:], lf[:], 0.0, None, Alu.add)

        # beta = clamp(t - left*h, 0, h)
        nc.vector.tensor_scalar(tlf[:], lf[:], float(h), None, Alu.mult)
        beta = small.tile([P, C], f32, tag="beta")
        nc.vector.tensor_tensor(beta[:], t_sb[:], tlf[:], Alu.subtract)
        nc.vector.tensor_scalar(beta[:], beta[:], 0.0, float(h), Alu.max, Alu.min)

        # deriv mask = (t > t0) & (t < tN)
        m_a = small.tile([P, C], f32, tag="ma")
        m_b = small.tile([P, C], f32, tag="mb")
        nc.vector.tensor_scalar(m_a[:], t_sb[:], t0, None, Alu.is_gt)
        nc.vector.tensor_scalar(m_b[:], t_sb[:], tN, None, Alu.is_lt)
        nc.vector.tensor_tensor(m_a[:], m_a[:], m_b[:], Alu.mult)

        # ---- merged 16-partition layouts + replication ----
        # x16[q, j*256+c] = x_nat[8q+j, c]  (dst AP splits free dim (j, c))
        idxs = wide.tile([P, M16], i16, tag="idxs")
        idxs_m = idxs[:1 * 16, :].rearrange("q (j c) -> q j c", j=8)
        nc.sync.dma_start(idxs_m, l_i16[:])
        for g in range(1, 8):
            nc.sync.dma_start(idxs[16 * g : 16 * (g + 1), :], idxs[0:16, :])

        # beta and mask share one [P, 2*M16] tile: [beta16 | mask16], so one
        # shifted copy per group serves both. The shift's cross-boundary
        # contamination lands in columns >= M16-8 of the beta half, which the
        # stride-8 scalar views never read.
        bm_sh = wide.tile([P, 2 * M16], f32, tag="bmsh")
        nc.sync.dma_start(bm_sh[0:16, 0:M16].rearrange("q (j c) -> q j c", j=8),
                          beta[:])
        nc.sync.dma_start(bm_sh[0:16, M16 : 2 * M16].rearrange("q (j c) -> q j c", j=8),
                          m_a[:])
        for g in range(1, 8):
            nc.sync.dma_start(bm_sh[16 * g : 16 * (g + 1), 0 : 2 * M16 - g],
                              bm_sh[0:16, g : 2 * M16])
        b_sh = bm_sh[:, 0:M16]
        m_sh = bm_sh[:, M16 : 2 * M16]

        # ---- gather + lerp per chunk ----
        for ch in range(NCHUNK):
            g3f = gat_p.tile([P, CCH * ROWB], u8, tag="gat")
            g3 = g3f[:].rearrange("p (c r) -> p c r", r=ROWB)
            nc.gpsimd.dma_gather(
                g3[:, :, :],
                tab[:, :],
                idxs[:, ch * (CCH * 8) : (ch + 1) * (CCH * 8)],
                num_idxs=CCH * P,
                num_idxs_reg=CCH * P,
                elem_size=ROWB,
                single_packet=False,
            )

            dtile = der_p.tile([P, CCH * DIM], f16, tag="der")
            d3 = dtile[:].rearrange("p (c d) -> p c d", d=DIM)
            vtile = der_p.tile([P, CCH * DIM], f16, tag="val")
            v3 = vtile[:].rearrange("p (c d) -> p c d", d=DIM)

            for j in range(CCH):
                c2 = ch * CCH + j
                pl = g3[:, j, 0 : 2 * DIM].bitcast(f16)
                dsc = g3[:, j, 2 * DIM : 4 * DIM].bitcast(f16)
                tmp = act_p.tile([P, DIM], f16, tag="tmp")
                bcol = b_sh[:, 8 * c2 : 8 * c2 + 1]
                if j % 2 == 1:  # balance: half the multiplies on DVE
                    nc.vector.tensor_scalar(tmp[:], dsc, bcol, None, Alu.mult)
                else:
                    nc.scalar.activation(tmp[:], dsc, Act.Copy, scale=bcol)
                nc.vector.tensor_tensor(v3[:, j, :], tmp[:], pl, Alu.add)
                nc.gpsimd.tensor_scalar(
                    d3[:, j, :], dsc, m_sh[:, 8 * c2 : 8 * c2 + 1], None, Alu.mult
                )

            cs = ch * CCH
            for g in range(8):
                ps = slice(16 * g, 16 * (g + 1))
                nc.sync.dma_start(val_v[g, :, cs : cs + CCH, :], v3[ps, :, :])
                nc.sync.dma_start(der_v[g, :, cs : cs + CCH, :], d3[ps, :, :])

    nc.compile()
    return nc


def _get_compiled(fixed_points, fixed_times, control_points, control_times):
    key = (fixed_times.tobytes(), control_times.tobytes(),
           fixed_points.tobytes(), control_points.tobytes())
    hit = _cache.get("k")
    if hit is not None and hit[0] == key:
        return hit[1], hit[2]
    all_times, table, h = _prep_tables(fixed_points, fixed_times,
                                       control_points, control_times)
    nc = _build(all_times, h)
    _cache["k"] = (key, nc, table)
    return nc, table


def kernel(t, fixed_points, fixed_times, control_points, control_times):
    from concourse.bass_utils import run_bass_kernel_spmd

    t = np.asarray(t, dtype=np.float32)
    if t.ndim == 2:
        t = t[:, 0]
    fixed_points = np.asarray(fixed_points, dtype=np.float32)
    fixed_times = np.asarray(fixed_times, dtype=np.float32)
    control_points = np.asarray(control_points, dtype=np.float32)
    control_times = np.asarray(control_times, dtype=np.float32)

    nc, table = _get_compiled(fixed_points, fixed_times,
                              control_points, control_times)

    shards = t.reshape(NCORES, BS)
    in_maps = [{"t_shard": np.ascontiguousarray(shards[i]), "table": table}
               for i in range(NCORES)]
    res = run_bass_kernel_spmd(nc, in_maps, core_ids=list(range(NCORES)))
    val = np.concatenate([r["val"] for r in res.results], axis=0).astype(np.float32)
    der = np.concatenate([r["der"] for r in res.results], axis=0).astype(np.float32)
    return val, der
